# revision 16
# baseline (speedup 1.0000x reference)
"""Trainium2 Bass kernel for BiDAF-style bidirectional attention.

Reference computation (per batch element n; M=1 folded away):
    s[i,j]  = h[i].w_h + u[j].w_u + (h[i]*u[j]).w_hu + b      [JX, JQ]
    a_u     = softmax_j(s);     u_a[i] = sum_j a_u[i,j] u[j]   (c2q)
    a_h     = softmax_i(max_j s);  h_a = sum_i a_h[i] h[i]     (q2c)
    out     = concat(h, u_a, h*u_a, h*h_a)                     [JX, 4D]

Sharding: data-parallel over batch N=8, one NeuronCore per batch element.
alpha_b drops out of the output entirely (it shifts s by a constant, and both
softmaxes are shift-invariant), so it is accepted but unused.

Per-core dataflow (i = context position, j = query position, d = feature):
  - h arrives [JX, D] row-major; matmuls contracting over d need h^T, built
    with 32 PE transposes (4 per PSUM bank, one batched ScalarE evict each).
  - scores are computed TRANSPOSED: s0T[j, i] = sum_d (u*w_hu)[j,d] h[i,d]
    via lhsT=uwT chunks, rhs=hT chunks, accumulating 4 d-chunks in PSUM.
    h.w_h is folded in with one extra K=1 matmul (ones_row outer hwh_row);
    u.w_u is folded in as the per-partition bias of the ScalarE Exp that
    evicts PSUM->SBUF: ET = exp(s0T + uwu[j]).  exp(hwh[i]) scales whole
    rows i of ET, which cancels in the j-softmax, and keeps max_j exact.
  - c2q: PE re-transposes ET (4 tiles per PSUM bank); one 3D DVE reduce per
    block gives row maxes/sums; u_a = (ET_tile^T @ u) scaled by 1/rowsum on
    DVE into a staging buffer shared with o3 = h*u_a (one DMA per tile).
  - q2c: weights w_i = max_j exp(...) = exp(max_j s - b); h_a via per-block
    M=1 f32r matmuls (block 0's overlap block 1's score work); broadcast
    back with a K=1 matmul; o4 muls split between DVE and GpSimd.
Engine balance: PE matmuls/transposes, ScalarE exp + PSUM evictions, DVE
reduces + normalize + output muls, GpSimd f32r copies + h passthrough DMAs.
A plain-f32 PE warmup burst (no cross-engine deps) lifts the HAM clock gate
to 2.4 GHz while the h DMAs are still in flight.
"""

import numpy as np

N_B, M_B, JX, JQ, D = 8, 1, 1024, 128, 512
P = 128
NT = JX // P   # 8 i-tiles
KC = D // P    # 4 d-chunks
IB = 512       # i-block width for score matmuls
NB = JX // IB  # 2 blocks
TPB = NT // NB  # tiles per block

_CACHE = {}


def _build_program():
    from contextlib import ExitStack

    import concourse.bass as bass
    import concourse.tile as tile
    from concourse import bacc, mybir
    from concourse.masks import make_identity

    f32 = mybir.dt.float32
    f32r = mybir.dt.float32r
    EXP = mybir.ActivationFunctionType.Exp
    AX = mybir.AxisListType.X
    MUL = mybir.AluOpType.mult
    ds = bass.ds

    nc = bacc.Bacc("TRN2", target_bir_lowering=False, debug=False, num_devices=8)
    h_d = nc.dram_tensor("h", [JX, D], f32, kind="ExternalInput").ap()
    u_d = nc.dram_tensor("u", [JQ, D], f32, kind="ExternalInput").ap()
    aw_d = nc.dram_tensor("alpha_w", [3 * D], f32, kind="ExternalInput").ap()
    out_d = nc.dram_tensor("out", [JX, 4 * D], f32, kind="ExternalOutput").ap()

    with tile.TileContext(nc) as tc, ExitStack() as ctx:
        consts = ctx.enter_context(tc.tile_pool(name="consts", bufs=1))
        stage = ctx.enter_context(tc.tile_pool(name="stage", bufs=6))
        # PSUM budget (8 banks): tp=2, s0=2, ua=2, acc=1, hap=1
        ps = ctx.enter_context(tc.tile_pool(name="ps", bufs=2, space="PSUM"))

        # ---- PE warmup: f32r N=512 matmuls depending only on DVE ops,
        # emitted first so the HAM clock-gate opens (1.2 -> 2.4 GHz) while
        # the h DMAs stream in (~630 ns each cold, ~6.3 us of PE busy).
        warm_f = consts.tile([P, D], f32)
        nc.vector.memset(warm_f[:], 0.25)
        warm = consts.tile([P, D], f32r)
        nc.vector.tensor_copy(warm[:], warm_f[:])
        wp = ps.tile([P, D], f32, tag="acc", bufs=1)
        for w in range(6):
            nc.tensor.matmul(
                wp[:], warm[:, ds(0, P)], warm[:], start=True, stop=True,
            )

        # ---- constants / prep ----
        ident = consts.tile([P, P], f32)
        make_identity(nc, ident[:])
        ident_r = consts.tile([P, P], f32r)
        nc.vector.tensor_copy(ident_r[:], ident[:])
        ones_row = consts.tile([1, P], f32)
        nc.vector.memset(ones_row[:], 1.0)
        ones_row_r = consts.tile([1, P], f32r)
        nc.scalar.copy(ones_row_r[:], ones_row[:])
        ones_col = consts.tile([P, 1], f32)
        nc.vector.memset(ones_col[:], 1.0)

        u_sb = consts.tile([JQ, D], f32)
        nc.sync.dma_start(u_sb[:], u_d[:])
        u_r = consts.tile([JQ, D], f32r)
        nc.scalar.copy(u_r[:], u_sb[:])
        w_cols = consts.tile([P, 12], f32)  # alpha_w partition-major: d = c*128+p
        nc.sync.dma_start(w_cols[:], aw_d.rearrange("(c p) -> p c", p=P))
        w_cols_r = consts.tile([P, 12], f32r)
        nc.vector.tensor_copy(w_cols_r[:], w_cols[:])
        whu_b = consts.tile([P, D], f32)  # w_hu broadcast across partitions
        nc.sync.dma_start(
            whu_b[:], aw_d[ds(2 * D, D)].rearrange("(o d) -> o d", o=1).to_broadcast((P, D))
        )
        wu_b = consts.tile([P, D], f32)  # w_u broadcast across partitions
        nc.sync.dma_start(
            wu_b[:], aw_d[ds(D, D)].rearrange("(o d) -> o d", o=1).to_broadcast((P, D))
        )

        # uw[j,d] = u[j,d]*w_hu[d];  uwu[j] = sum_d u[j,d]*w_u[d]
        uw = consts.tile([JQ, D], f32)
        nc.vector.tensor_mul(uw[:], u_sb[:], whu_b[:])
        uwtmp = consts.tile([JQ, D], f32)
        nc.vector.tensor_mul(uwtmp[:], u_sb[:], wu_b[:])
        uwu = consts.tile([JQ, 1], f32)
        nc.vector.reduce_sum(uwu[:], uwtmp[:], axis=AX)

        # uwT[d_chunk][j]: 4 transposes into one PSUM bank, one batched evict
        uwT = consts.tile([P, KC * JQ], f32r)
        pt = ps.tile([P, KC * P], f32, tag="tp")
        for k in range(KC):
            nc.tensor.transpose(pt[:, ds(k * P, P)], uw[:, ds(k * P, P)], ident[:])
        nc.scalar.copy(uwT[:], pt[:])

        # ---- load h; h_r (f32r copy on GpSimd, for q2c matmuls); build hT ----
        h_all = consts.tile([P, NT * D], f32)    # tile t: h[t*128+p, d]
        h_r = consts.tile([P, NT * D], f32r)
        hT_all = consts.tile([P, KC * JX], f32r)  # chunk k: hT[k*128+p, i]
        hT3 = hT_all[:].rearrange("p (k x) -> p k x", k=KC)
        for t in range(NT):
            # two DMAs per tile on different queues for issue parallelism
            nc.sync.dma_start(h_all[:, ds(t * D, D // 2)], h_d[ds(t * P, P), ds(0, D // 2)])
            nc.gpsimd.dma_start(
                h_all[:, ds(t * D + D // 2, D // 2)], h_d[ds(t * P, P), ds(D // 2, D // 2)]
            )
            # out1 = h passthrough (GpSimd DMA queue; Sync stays free)
            nc.gpsimd.dma_start(out_d[ds(t * P, P), ds(0, D)], h_all[:, ds(t * D, D)])
        for t in range(NT):
            pt = ps.tile([P, KC * P], f32, tag="tp")
            for k in range(KC):
                nc.tensor.transpose(
                    pt[:, ds(k * P, P)], h_all[:, ds(t * D + k * P, P)], ident[:]
                )
            ev = nc.scalar.copy if t % 2 == 0 else nc.vector.tensor_copy
            ev(hT3[:, :, ds(t * P, P)], pt[:].rearrange("p (k x) -> p k x", k=KC))

        # ---- scores (transposed), exp, c2q, per-block q2c accumulation ----
        hwh_row = consts.tile([1, JX], f32r)      # h.w_h as a row over i
        ET = consts.tile([JQ, JX], f32r)          # exp(s0T + uwu[j]) (row-scaled)
        m_exp = consts.tile([P, NT], f32)         # per i-tile: max_j ET
        m_exp_r = consts.tile([P, NT], f32r)
        z_rec = consts.tile([P, NT], f32)         # per i-tile: 1/sum_j ET
        hap = ps.tile([1, D], f32, tag="hap", bufs=1)

        for b in range(NB):
            blk = ds(b * IB, IB)
            for q in range(TPB):
                t = b * TPB + q
                nc.scalar.copy(h_r[:, ds(t * D, D)], h_all[:, ds(t * D, D)])
            # hwh chunk: [1, IB] row accumulated over d-chunks
            hp = ps.tile([1, IB], f32, tag="acc", bufs=1)
            for k in range(KC):
                nc.tensor.matmul(
                    hp[:], w_cols_r[:, ds(k, 1)], hT_all[:, ds(k * JX + b * IB, IB)],
                    start=(k == 0), stop=(k == KC - 1),
                )
            nc.scalar.copy(hwh_row[:, blk], hp[:])

            sp = ps.tile([JQ, IB], f32, tag="s0")
            for k in range(KC):
                nc.tensor.matmul(
                    sp[:], uwT[:, ds(k * JQ, JQ)], hT_all[:, ds(k * JX + b * IB, IB)],
                    start=(k == 0), stop=False,
                )
            nc.tensor.matmul(
                sp[:], ones_row_r[:], hwh_row[:, blk], start=False, stop=True
            )
            # ET = exp(s0T + uwu[j]); uwu is the per-partition (j) ACT bias
            nc.scalar.activation(ET[:, blk], sp[:], EXP, bias=uwu[:])

            # re-transpose ET (4 tiles into one bank); batched 3D reduces
            et = ps.tile([P, TPB * P], f32r, tag="tp")
            for q in range(TPB):
                t = b * TPB + q
                nc.tensor.transpose(
                    et[:, ds(q * P, P)], ET[:, ds(t * P, P)], ident_r[:]
                )
            et3 = et[:].rearrange("p (q x) -> p q x", q=TPB)
            nc.vector.reduce_max(m_exp[:, ds(b * TPB, TPB)], et3, axis=AX)
            zsum = stage.tile([P, TPB], f32, tag="zs")
            nc.vector.reduce_sum(zsum[:], et3, axis=AX)
            nc.vector.reciprocal(z_rec[:, ds(b * TPB, TPB)], zsum[:])
            nc.scalar.copy(m_exp_r[:, ds(b * TPB, TPB)], m_exp[:, ds(b * TPB, TPB)])

            # q2c accumulation for this block's tiles (single PSUM group
            # spanning both blocks; other matmuls interleave freely)
            for q in range(TPB):
                t = b * TPB + q
                nc.tensor.matmul(
                    hap[:], m_exp_r[:, ds(t, 1)], h_r[:, ds(t * D, D)],
                    start=(b == 0 and q == 0), stop=(b == NB - 1 and q == TPB - 1),
                    skip_group_check=True,
                )
            if b == NB - 1:
                # q2c chain emitted ahead of the last c2q loop: bc becomes
                # ready while stg work still streams, shortening the tail
                mrow = consts.tile([P, 1], f32)
                nc.vector.reduce_sum(mrow[:], m_exp[:], axis=AX)
                zqp = ps.tile([1, 1], f32, tag="acc", bufs=1)
                nc.tensor.matmul(zqp[:], mrow[:], ones_col[:], start=True, stop=True)
                rzq = consts.tile([1, 1], f32)
                nc.vector.reciprocal(rzq[:], zqp[:])
                ha_sum = consts.tile([1, D], f32)
                nc.vector.tensor_copy(ha_sum[:], hap[:])
                ha_row = consts.tile([1, D], f32r)
                nc.scalar.mul(ha_row[:], ha_sum[:], rzq[:])
                bc = ps.tile([P, D], f32, tag="acc", bufs=1)
                nc.tensor.matmul(bc[:], ones_row_r[:], ha_row[:], start=True, stop=True)

            for q in range(TPB):
                t = b * TPB + q
                up = ps.tile([P, D], f32, tag="ua")
                nc.tensor.matmul(
                    up[:], ET[:, ds(t * P, P)], u_r[:], start=True, stop=True
                )
                stg = stage.tile([P, 2 * D], f32, tag="stg")
                nc.scalar.mul(stg[:, ds(0, D)], up[:], z_rec[:, ds(t, 1)])
                nc.vector.tensor_mul(
                    stg[:, ds(D, D)], h_all[:, ds(t * D, D)], stg[:, ds(0, D)]
                )
                nc.sync.dma_start(out_d[ds(t * P, P), ds(D, 2 * D)], stg[:])
                if b == NB - 1:
                    # interleave two o4 tiles after each stg tile
                    for tt in (2 * q, 2 * q + 1):
                        o4 = stage.tile([P, D], f32, tag="o4")
                        nc.vector.tensor_mul(o4[:], h_all[:, ds(tt * D, D)], bc[:])
                        eng = nc.sync if tt % 2 == 0 else nc.gpsimd
                        eng.dma_start(out_d[ds(tt * P, P), ds(3 * D, D)], o4[:])

    nc.compile()
    return nc


def _get_nc():
    if "nc" not in _CACHE:
        _CACHE["nc"] = _build_program()
    return _CACHE["nc"]


def kernel(h, u, alpha_w, alpha_b=None, **_unused):
    from concourse.bass_utils import run_bass_kernel_spmd

    h = np.ascontiguousarray(np.asarray(h, dtype=np.float32)).reshape(N_B, JX, D)
    u = np.ascontiguousarray(np.asarray(u, dtype=np.float32)).reshape(N_B, JQ, D)
    alpha_w = np.ascontiguousarray(np.asarray(alpha_w, dtype=np.float32)).reshape(3 * D)

    nc = _get_nc()
    in_maps = [
        {"h": h[n], "u": u[n], "alpha_w": alpha_w} for n in range(N_B)
    ]
    res = run_bass_kernel_spmd(nc, in_maps, core_ids=list(range(N_B)))
    out = np.stack([res.results[n]["out"] for n in range(N_B)], axis=0)
    return out.reshape(N_B, M_B, JX, 4 * D)


# revision 18
# speedup vs baseline: 1.0943x; 1.0943x over previous
"""Trainium2 Bass kernel for BiDAF-style bidirectional attention.

Reference computation (per batch element n; M=1 folded away):
    s[i,j]  = h[i].w_h + u[j].w_u + (h[i]*u[j]).w_hu + b      [JX, JQ]
    a_u     = softmax_j(s);     u_a[i] = sum_j a_u[i,j] u[j]   (c2q)
    a_h     = softmax_i(max_j s);  h_a = sum_i a_h[i] h[i]     (q2c)
    out     = concat(h, u_a, h*u_a, h*h_a)                     [JX, 4D]

Sharding: data-parallel over batch N=8, one NeuronCore per batch element.
alpha_b drops out of the output entirely (it shifts s by a constant, and both
softmaxes are shift-invariant), so it is accepted but unused.

Per-core dataflow (i = context position, j = query position, d = feature):
  - h arrives [JX, D] row-major; matmuls contracting over d need h^T, built
    with 32 PE transposes (4 per PSUM bank, one batched ScalarE evict each).
  - scores are computed TRANSPOSED: s0T[j, i] = sum_d (u*w_hu)[j,d] h[i,d]
    via lhsT=uwT chunks, rhs=hT chunks, accumulating 4 d-chunks in PSUM.
    h.w_h is folded in with one extra K=1 matmul (ones_row outer hwh_row);
    u.w_u is folded in as the per-partition bias of the ScalarE Exp that
    evicts PSUM->SBUF: ET = exp(s0T + uwu[j]).  exp(hwh[i]) scales whole
    rows i of ET, which cancels in the j-softmax, and keeps max_j exact.
  - c2q: PE re-transposes ET (4 tiles per PSUM bank); one 3D DVE reduce per
    block gives row maxes/sums; u_a = (ET_tile^T @ u) scaled by 1/rowsum on
    DVE into a staging buffer shared with o3 = h*u_a (one DMA per tile).
  - q2c: weights w_i = max_j exp(...) = exp(max_j s - b); h_a via per-block
    M=1 f32r matmuls (block 0's overlap block 1's score work); broadcast
    back with a K=1 matmul; o4 muls split between DVE and GpSimd.
Engine balance: PE matmuls/transposes, ScalarE exp + PSUM evictions, DVE
reduces + normalize + output muls, GpSimd f32r copies + h passthrough DMAs.
A plain-f32 PE warmup burst (no cross-engine deps) lifts the HAM clock gate
to 2.4 GHz while the h DMAs are still in flight.
"""

import numpy as np

N_B, M_B, JX, JQ, D = 8, 1, 1024, 128, 512
P = 128
NT = JX // P   # 8 i-tiles
KC = D // P    # 4 d-chunks
IB = 512       # i-block width for score matmuls
NB = JX // IB  # 2 blocks
TPB = NT // NB  # tiles per block

_CACHE = {}


def _build_program():
    from contextlib import ExitStack

    import concourse.bass as bass
    import concourse.tile as tile
    from concourse import bacc, mybir
    from concourse.masks import make_identity

    f32 = mybir.dt.float32
    f32r = mybir.dt.float32r
    EXP = mybir.ActivationFunctionType.Exp
    AX = mybir.AxisListType.X
    MUL = mybir.AluOpType.mult
    ds = bass.ds

    nc = bacc.Bacc("TRN2", target_bir_lowering=False, debug=False, num_devices=8)
    h_d = nc.dram_tensor("h", [JX, D], f32, kind="ExternalInput").ap()
    u_d = nc.dram_tensor("u", [JQ, D], f32, kind="ExternalInput").ap()
    aw_d = nc.dram_tensor("alpha_w", [3 * D], f32, kind="ExternalInput").ap()
    out_d = nc.dram_tensor("out", [JX, 4 * D], f32, kind="ExternalOutput").ap()

    with tile.TileContext(nc) as tc, ExitStack() as ctx:
        consts = ctx.enter_context(tc.tile_pool(name="consts", bufs=1))
        stage = ctx.enter_context(tc.tile_pool(name="stage", bufs=6))
        # PSUM budget (8 banks): tp=2, s0=2, ua=2, acc=1, hap=1
        ps = ctx.enter_context(tc.tile_pool(name="ps", bufs=2, space="PSUM"))

        # ---- PE warmup: f32r N=512 matmuls depending only on DVE ops,
        # emitted first so the HAM clock-gate opens (1.2 -> 2.4 GHz) while
        # the h DMAs stream in (~630 ns each cold, ~6.3 us of PE busy).
        warm_f = consts.tile([P, D], f32)
        nc.vector.memset(warm_f[:], 0.25)
        warm = consts.tile([P, D], f32r)
        nc.vector.tensor_copy(warm[:], warm_f[:])
        wp = ps.tile([P, D], f32, tag="acc", bufs=1)
        for w in range(8):
            nc.tensor.matmul(
                wp[:], warm[:, ds(0, P)], warm[:], start=True, stop=True,
            )

        # ---- constants / prep ----
        ident = consts.tile([P, P], f32)
        make_identity(nc, ident[:])
        ident_r = consts.tile([P, P], f32r)
        nc.vector.tensor_copy(ident_r[:], ident[:])
        ones_row = consts.tile([1, P], f32)
        nc.vector.memset(ones_row[:], 1.0)
        ones_row_r = consts.tile([1, P], f32r)
        nc.scalar.copy(ones_row_r[:], ones_row[:])
        ones_col = consts.tile([P, 1], f32)
        nc.vector.memset(ones_col[:], 1.0)

        u_sb = consts.tile([JQ, D], f32)
        nc.sync.dma_start(u_sb[:], u_d[:])
        u_r = consts.tile([JQ, D], f32r)
        nc.scalar.copy(u_r[:], u_sb[:])
        w_cols = consts.tile([P, 12], f32)  # alpha_w partition-major: d = c*128+p
        nc.sync.dma_start(w_cols[:], aw_d.rearrange("(c p) -> p c", p=P))
        w_cols_r = consts.tile([P, 12], f32r)
        nc.vector.tensor_copy(w_cols_r[:], w_cols[:])
        whu_b = consts.tile([P, D], f32)  # w_hu broadcast across partitions
        nc.sync.dma_start(
            whu_b[:], aw_d[ds(2 * D, D)].rearrange("(o d) -> o d", o=1).to_broadcast((P, D))
        )
        wu_b = consts.tile([P, D], f32)  # w_u broadcast across partitions
        nc.sync.dma_start(
            wu_b[:], aw_d[ds(D, D)].rearrange("(o d) -> o d", o=1).to_broadcast((P, D))
        )

        # uw[j,d] = u[j,d]*w_hu[d];  uwu[j] = sum_d u[j,d]*w_u[d]
        uw = consts.tile([JQ, D], f32)
        nc.vector.tensor_mul(uw[:], u_sb[:], whu_b[:])
        uwtmp = consts.tile([JQ, D], f32)
        nc.vector.tensor_mul(uwtmp[:], u_sb[:], wu_b[:])
        uwu = consts.tile([JQ, 1], f32)
        nc.vector.reduce_sum(uwu[:], uwtmp[:], axis=AX)

        # uwT[d_chunk][j]: 4 transposes into one PSUM bank, one batched evict
        uwT = consts.tile([P, KC * JQ], f32r)
        pt = ps.tile([P, KC * P], f32, tag="tp")
        for k in range(KC):
            nc.tensor.transpose(pt[:, ds(k * P, P)], uw[:, ds(k * P, P)], ident[:])
        nc.scalar.copy(uwT[:], pt[:])

        # ---- load h; h_r (f32r copy on GpSimd, for q2c matmuls); build hT ----
        h_all = consts.tile([P, NT * D], f32)    # tile t: h[t*128+p, d]
        h_r = consts.tile([P, NT * D], f32r)
        hT_all = consts.tile([P, KC * JX], f32r)  # chunk k: hT[k*128+p, i]
        hT3 = hT_all[:].rearrange("p (k x) -> p k x", k=KC)
        for t in range(NT):
            nc.sync.dma_start(h_all[:, ds(t * D, D)], h_d[ds(t * P, P), :])
            # out1 = h passthrough (GpSimd DMA queue; Sync stays free)
            nc.gpsimd.dma_start(out_d[ds(t * P, P), ds(0, D)], h_all[:, ds(t * D, D)])
        for t in range(NT):
            pt = ps.tile([P, KC * P], f32, tag="tp")
            for k in range(KC):
                nc.tensor.transpose(
                    pt[:, ds(k * P, P)], h_all[:, ds(t * D + k * P, P)], ident[:]
                )
            ev = nc.scalar.copy if t % 2 == 0 else nc.vector.tensor_copy
            ev(hT3[:, :, ds(t * P, P)], pt[:].rearrange("p (k x) -> p k x", k=KC))

        # ---- scores (transposed), exp, c2q, per-block q2c accumulation ----
        hwh_row = consts.tile([1, JX], f32r)      # h.w_h as a row over i
        ET = consts.tile([JQ, JX], f32r)          # exp(s0T + uwu[j]) (row-scaled)
        m_exp = consts.tile([P, NT], f32)         # per i-tile: max_j ET
        m_exp_r = consts.tile([P, NT], f32r)
        z_rec = consts.tile([P, NT], f32)         # per i-tile: 1/sum_j ET
        hap = ps.tile([1, D], f32, tag="hap", bufs=1)

        for b in range(NB):
            blk = ds(b * IB, IB)
            for q in range(TPB):
                t = b * TPB + q
                nc.scalar.copy(h_r[:, ds(t * D, D)], h_all[:, ds(t * D, D)])
            # hwh chunk: [1, IB] row accumulated over d-chunks
            hp = ps.tile([1, IB], f32, tag="acc", bufs=1)
            for k in range(KC):
                nc.tensor.matmul(
                    hp[:], w_cols_r[:, ds(k, 1)], hT_all[:, ds(k * JX + b * IB, IB)],
                    start=(k == 0), stop=(k == KC - 1),
                )
            nc.scalar.copy(hwh_row[:, blk], hp[:])

            sp = ps.tile([JQ, IB], f32, tag="s0")
            for k in range(KC):
                nc.tensor.matmul(
                    sp[:], uwT[:, ds(k * JQ, JQ)], hT_all[:, ds(k * JX + b * IB, IB)],
                    start=(k == 0), stop=False,
                )
            nc.tensor.matmul(
                sp[:], ones_row_r[:], hwh_row[:, blk], start=False, stop=True
            )
            # ET = exp(s0T + uwu[j]); uwu is the per-partition (j) ACT bias
            nc.scalar.activation(ET[:, blk], sp[:], EXP, bias=uwu[:])

            # re-transpose ET (4 tiles into one bank); batched 3D reduces
            et = ps.tile([P, TPB * P], f32r, tag="tp")
            for q in range(TPB):
                t = b * TPB + q
                nc.tensor.transpose(
                    et[:, ds(q * P, P)], ET[:, ds(t * P, P)], ident_r[:]
                )
            et3 = et[:].rearrange("p (q x) -> p q x", q=TPB)
            nc.vector.reduce_max(m_exp[:, ds(b * TPB, TPB)], et3, axis=AX)
            zsum = stage.tile([P, TPB], f32, tag="zs")
            nc.vector.reduce_sum(zsum[:], et3, axis=AX)
            nc.vector.reciprocal(z_rec[:, ds(b * TPB, TPB)], zsum[:])
            nc.scalar.copy(m_exp_r[:, ds(b * TPB, TPB)], m_exp[:, ds(b * TPB, TPB)])

            # q2c accumulation for this block's tiles (single PSUM group
            # spanning both blocks; other matmuls interleave freely)
            for q in range(TPB):
                t = b * TPB + q
                nc.tensor.matmul(
                    hap[:], m_exp_r[:, ds(t, 1)], h_r[:, ds(t * D, D)],
                    start=(b == 0 and q == 0), stop=(b == NB - 1 and q == TPB - 1),
                    skip_group_check=True,
                )
            if b == NB - 1:
                # q2c chain emitted ahead of the last c2q loop: bc becomes
                # ready while stg work still streams, shortening the tail
                mrow = consts.tile([P, 1], f32)
                nc.vector.reduce_sum(mrow[:], m_exp[:], axis=AX)
                zqp = ps.tile([1, 1], f32, tag="acc", bufs=1)
                nc.tensor.matmul(zqp[:], mrow[:], ones_col[:], start=True, stop=True)
                rzq = consts.tile([1, 1], f32)
                nc.vector.reciprocal(rzq[:], zqp[:])
                ha_sum = consts.tile([1, D], f32)
                nc.vector.tensor_copy(ha_sum[:], hap[:])
                ha_row = consts.tile([1, D], f32r)
                nc.scalar.mul(ha_row[:], ha_sum[:], rzq[:])
                bc = ps.tile([P, D], f32, tag="acc", bufs=1)
                nc.tensor.matmul(bc[:], ones_row_r[:], ha_row[:], start=True, stop=True)

            for q in range(TPB):
                t = b * TPB + q
                up = ps.tile([P, D], f32, tag="ua")
                nc.tensor.matmul(
                    up[:], ET[:, ds(t * P, P)], u_r[:], start=True, stop=True
                )
                stg = stage.tile([P, 2 * D], f32, tag="stg")
                nc.scalar.mul(stg[:, ds(0, D)], up[:], z_rec[:, ds(t, 1)])
                nc.vector.tensor_mul(
                    stg[:, ds(D, D)], h_all[:, ds(t * D, D)], stg[:, ds(0, D)]
                )
                nc.sync.dma_start(out_d[ds(t * P, P), ds(D, 2 * D)], stg[:])
                if b == NB - 1:
                    # interleave two o4 tiles after each stg tile
                    for tt in (2 * q, 2 * q + 1):
                        o4 = stage.tile([P, D], f32, tag="o4")
                        nc.vector.tensor_mul(o4[:], h_all[:, ds(tt * D, D)], bc[:])
                        eng = nc.sync if tt % 2 == 0 else nc.gpsimd
                        eng.dma_start(out_d[ds(tt * P, P), ds(3 * D, D)], o4[:])

    nc.compile()
    return nc


def _get_nc():
    if "nc" not in _CACHE:
        _CACHE["nc"] = _build_program()
    return _CACHE["nc"]


def kernel(h, u, alpha_w, alpha_b=None, **_unused):
    from concourse.bass_utils import run_bass_kernel_spmd

    h = np.ascontiguousarray(np.asarray(h, dtype=np.float32)).reshape(N_B, JX, D)
    u = np.ascontiguousarray(np.asarray(u, dtype=np.float32)).reshape(N_B, JQ, D)
    alpha_w = np.ascontiguousarray(np.asarray(alpha_w, dtype=np.float32)).reshape(3 * D)

    nc = _get_nc()
    in_maps = [
        {"h": h[n], "u": u[n], "alpha_w": alpha_w} for n in range(N_B)
    ]
    res = run_bass_kernel_spmd(nc, in_maps, core_ids=list(range(N_B)))
    out = np.stack([res.results[n]["out"] for n in range(N_B)], axis=0)
    return out.reshape(N_B, M_B, JX, 4 * D)
